# revision 1
# baseline (speedup 1.0000x reference)
"""Single-qubit Kraus channel on a batched density matrix, on 8 trn2 cores.

rho -> sum_k K_k rho K_k^dagger applied to one target qubit of an n-qubit
density matrix state[2^n, 2^n, B].

The two-sided contraction reduces to a 4x4 coefficient matrix
    C[p,q,i,j] = sum_k K[k,p,i] * conj(K[k,q,j])
acting block-wise: out(row-bit p, col-bit q) = sum_ij C[p,q,i,j] * in(i, j),
i.e. every output element is a <=4-term linear combination of input elements
that differ only in the target bit of the row/column index.  Pure memory
bound: read input once, write output once.

Sharding: data-parallel over contiguous row blocks (4096 rows -> 512/core).
Per core, tiles of [128 partitions x 4096 f32] pair the target-row-bit
halves on identical partitions so all compute is lane-aligned:
  partition p = a_local*64 + r  <->  dram row g*256 + a_local*128 + i*64 + r
Compute per output block: ScalarE scaled copy for the smallest term,
VectorE scalar_tensor_tensor (fused (x*c)+y) for the rest.
"""

import sys

import numpy as np

try:
    import concourse.bass  # noqa: F401  (resolves via the default env path)
except ImportError:
    _REPO = "/opt/trn_rl_repo"
    if _REPO not in sys.path:
        sys.path.insert(0, _REPO)

import concourse.bacc as bacc
import concourse.bass as bass
import concourse.mybir as mybir
from concourse.bass_utils import run_bass_kernel_spmd
from concourse.tile import TileContext

N_CORES = 8

# Graded configuration (reference.setup_inputs): n=12, target=5, B=4.
DIM = 4096
BATCH = 4
ROWS = DIM // N_CORES          # 512 rows per core
FREE = DIM * BATCH             # 16384 f32 per row
R_ROW = 64                     # rows right of target bit (row side)
RB = R_ROW * BATCH             # 256 f32: one col-side j-block
CGRP = 2 * RB                  # 512 f32: one col group (j=0 half + j=1 half)
W = 8192                       # chunk: f32 per partition per tile (16 col groups)
NW = FREE // W                 # 4 chunks
NG = ROWS // (4 * R_ROW)       # 2 supergroups of 256 rows (two 128-row a-groups)

_COEF_TOL = 0.0  # exact-zero test; bit-flip channel cross terms are exact 0s

_prog_cache: dict = {}


def _build_program(
    coefs: tuple,
    repeat: int = 1,
    tile_w: int = W,
    store_engine: str = "sync",
    bufs: int = 3,
    obufs: int | None = None,
    paired: bool = False,
) -> "bass.Bass":
    """Build the per-core SPMD program for coefficient matrix C[p,q,i,j].

    repeat > 1 wraps the whole body in a hardware loop — benchmarking only
    (recomputes the same output repeat times).
    """
    f32 = mybir.dt.float32
    W_ = tile_w
    NW_ = FREE // W_
    n_agrp = ROWS // 128  # natural 128-row groups per core

    nc = bacc.Bacc("TRN2", target_bir_lowering=False, debug=False)
    x = nc.dram_tensor("x", [ROWS, FREE], f32, kind="ExternalInput")
    y = nc.dram_tensor("y", [ROWS, FREE], f32, kind="ExternalOutput")

    def pjview(tile, p, j):
        # [64, ncg, RB]: partition half p (row target-bit), col-side j half
        # of every col group.
        return tile[p * 64 : (p + 1) * 64].rearrange(
            "p (c j t) -> p c j t", j=2, t=RB
        )[:, :, j, :]

    def terms_for(p, q):
        terms = [
            (coefs[((p * 2 + q) * 2 + i) * 2 + j], i, j)
            for i in (0, 1)
            for j in (0, 1)
            if abs(coefs[((p * 2 + q) * 2 + i) * 2 + j]) > _COEF_TOL
        ]
        terms.sort(key=lambda it: -abs(it[0]))
        return terms

    def emit_block(ov, xview, p, q, aligned_pred, scratch_view=None):
        # One ScalarE scaled copy seeds ov; remaining terms accumulate in
        # place via VectorE fused (x*c)+y.  In-place keeps each
        # instruction's semaphore-wait count low (the STT encoding has few
        # sync-wait slots).  HW constraint: STT's two SBUF inputs must
        # share a base partition, so terms whose source partition half
        # differs from ov's ("cross" terms) can only ride ScalarE (1-input,
        # cross-offset allowed) or accumulate in scratch at their own base.
        terms = terms_for(p, q)
        if not terms:
            nc.vector.memset(ov, 0.0)
            return
        aligned = [t for t in terms if aligned_pred(t[1])]
        cross = [t for t in terms if not aligned_pred(t[1])]
        if not cross:
            c0, i0, j0 = aligned[0]
            nc.scalar.mul(ov, xview(i0, j0), c0)
            rest = aligned[1:]
        elif len(cross) == 1:
            c0, i0, j0 = cross[0]
            nc.scalar.mul(ov, xview(i0, j0), c0)
            rest = aligned
        else:
            s = scratch_view(1 - p)
            c0, i0, j0 = cross[0]
            nc.scalar.mul(s, xview(i0, j0), c0)
            for ck, ik, jk in cross[1:]:
                nc.vector.scalar_tensor_tensor(
                    out=s,
                    in0=xview(ik, jk),
                    scalar=float(ck),
                    in1=s,
                    op0=mybir.AluOpType.mult,
                    op1=mybir.AluOpType.add,
                )
            nc.scalar.mul(ov, s, 1.0)
            rest = aligned
        for ck, ik, jk in rest:
            nc.vector.scalar_tensor_tensor(
                out=ov,
                in0=xview(ik, jk),
                scalar=float(ck),
                in1=ov,
                op0=mybir.AluOpType.mult,
                op1=mybir.AluOpType.add,
            )

    def jview128(tile, j):
        # [128, ncg, RB]: col-side j half of every col group, all partitions
        return tile.rearrange("p (c j t) -> p c j t", j=2, t=RB)[:, :, j, :]

    from contextlib import ExitStack

    if paired:
        with TileContext(nc) as tc, ExitStack() as stack:
            if repeat > 1:
                stack.enter_context(tc.For_i(0, repeat, 1))
            with tc.tile_pool(name="xin", bufs=bufs) as px, \
                 tc.tile_pool(name="yout", bufs=obufs or bufs) as po:
                for b in range(ROWS // 256):
                    r0 = b * 256
                    for w in range(NW_):
                        cs = slice(w * W_, (w + 1) * W_)
                        xt = []
                        for i in (0, 1):
                            t = px.tile([128, W_], f32, tag=f"x{i}")
                            nc.sync.dma_start(
                                out=t[0:64],
                                in_=x[r0 + i * 64 : r0 + i * 64 + 64, cs],
                            )
                            nc.sync.dma_start(
                                out=t[64:128],
                                in_=x[r0 + 128 + i * 64 : r0 + 128 + i * 64 + 64, cs],
                            )
                            xt.append(t)
                        for p in (0, 1):
                            ot = po.tile([128, W_], f32, tag=f"o{p}")
                            for q in (0, 1):
                                emit_block(
                                    jview128(ot, q),
                                    lambda i, j: jview128(xt[i], j),
                                    p,
                                    q,
                                    aligned_pred=lambda i: True,
                                )
                            eng = getattr(nc, store_engine)
                            eng.dma_start(
                                out=y[r0 + p * 64 : r0 + p * 64 + 64, cs],
                                in_=ot[0:64],
                            )
                            eng.dma_start(
                                out=y[r0 + 128 + p * 64 : r0 + 128 + p * 64 + 64, cs],
                                in_=ot[64:128],
                            )
        nc.compile()
        return nc

    # does any block route >=2 cross-partition terms through scratch?
    needs_scratch = any(
        len([t for t in terms_for(p, q) if t[1] != p]) >= 2
        for p in (0, 1)
        for q in (0, 1)
    )

    # scratch costs SBUF: drop to double buffering to stay within 224 KiB
    if needs_scratch:
        bufs = min(bufs, 2)

    with TileContext(nc) as tc, ExitStack() as stack:
        if repeat > 1:
            stack.enter_context(tc.For_i(0, repeat, 1))
        with tc.tile_pool(name="xin", bufs=bufs) as px, \
             tc.tile_pool(name="yout", bufs=obufs or bufs) as po, \
             tc.tile_pool(name="scr", bufs=2) as ps:
            for a in range(n_agrp):
                rs = slice(a * 128, (a + 1) * 128)
                for w in range(NW_):
                    cs = slice(w * W_, (w + 1) * W_)
                    xt = px.tile([128, W_], f32, tag="x")
                    # 128 consecutive DRAM rows -> 128 partitions; fully
                    # contiguous 32 KiB runs per partition (fast DMA path).
                    # Partitions 0-63 hold target-row-bit 0, 64-127 bit 1.
                    nc.sync.dma_start(out=xt[:], in_=x[rs, cs])
                    ot = po.tile([128, W_], f32, tag="o")
                    for p in (0, 1):
                        if needs_scratch:
                            st = ps.tile([128, W_ // 2], f32, tag="s")

                            def scratch_view(half, _st=st):
                                return _st[
                                    half * 64 : (half + 1) * 64
                                ].rearrange("p (c t) -> p c t", t=RB)
                        else:
                            scratch_view = None
                        for q in (0, 1):
                            # Reads with i != p are cross-partition-offset
                            # (supported on ScalarE; the STT same-base
                            # constraint is handled in emit_block).
                            emit_block(
                                pjview(ot, p, q),
                                lambda i, j: pjview(xt, i, j),
                                p,
                                q,
                                aligned_pred=lambda i, _p=p: i == _p,
                                scratch_view=scratch_view,
                            )
                    getattr(nc, store_engine).dma_start(
                        out=y[rs, cs], in_=ot[:]
                    )
    nc.compile()
    return nc


def _fallback(state, C, L, R, B):
    rho = state.reshape(L, 2, R, L, 2, R, B)
    out = np.einsum("pqij,aibcjdz->apbcqdz", C, rho.astype(np.float64))
    return out.reshape(state.shape).astype(state.dtype)


def kernel(state, kraus, target, n_qubits):
    state = np.asarray(state)
    kraus = np.asarray(kraus)
    t = int(np.asarray(target))
    n = int(np.asarray(n_qubits))
    dim = 1 << n
    B = state.shape[-1]
    L = 1 << t
    R = dim // (2 * L)

    C = np.einsum(
        "kpi,kqj->pqij",
        kraus.astype(np.float64),
        np.conj(kraus).astype(np.float64),
    )

    if not (
        state.shape == (DIM, DIM, BATCH)
        and state.dtype == np.float32
        and R == R_ROW
        and L * 2 * R == DIM
    ):
        return _fallback(state, C, L, R, B)

    coefs = tuple(float(v) for v in C.reshape(-1))
    nc = _prog_cache.get(coefs)
    if nc is None:
        nc = _build_program(coefs)
        _prog_cache[coefs] = nc

    flat = state.reshape(DIM, FREE)
    in_maps = [
        {"x": flat[c * ROWS : (c + 1) * ROWS]} for c in range(N_CORES)
    ]
    res = run_bass_kernel_spmd(nc, in_maps, core_ids=list(range(N_CORES)))
    out = np.concatenate([res.results[c]["y"] for c in range(N_CORES)], axis=0)
    return out.reshape(DIM, DIM, BATCH)



# revision 4
# speedup vs baseline: 1.0573x; 1.0573x over previous
"""Single-qubit Kraus channel on a batched density matrix, on 8 trn2 cores.

rho -> sum_k K_k rho K_k^dagger applied to one target qubit of an n-qubit
density matrix state[2^n, 2^n, B].

The two-sided contraction reduces to a 4x4 coefficient matrix
    C[p,q,i,j] = sum_k K[k,p,i] * conj(K[k,q,j])
acting block-wise: out(row-bit p, col-bit q) = sum_ij C[p,q,i,j] * in(i, j),
i.e. every output element is a <=4-term linear combination of input elements
that differ only in the target bit of the row/column index.  Pure memory
bound: read input once, write output once.

Sharding: data-parallel over contiguous row blocks (4096 rows -> 512/core).

Graded (bit-flip-structured) path — fused full-row program: per
[128, W] tile the whole update is
  2 seeds  (64-wide full rows: the cross term reads the other partition
            half through a j-reversed negative-stride view; the second
            seed alternates ScalarE/VectorE per tile to balance engines)
  1 STT    (128-wide full row, dense: the aligned 0.9*x term)
which cuts instruction count ~4x vs blockwise and balances ACT/DVE busy
at ~110 us each, under the ~186 us/core HBM stream (memcpy-measured
roofline ~189 us).  Generic channels fall back to the blockwise program.
"""

import sys

import numpy as np

try:
    import concourse.bass  # noqa: F401  (resolves via the default env path)
except ImportError:
    _REPO = "/opt/trn_rl_repo"
    if _REPO not in sys.path:
        sys.path.insert(0, _REPO)

import concourse.bacc as bacc
import concourse.bass as bass
import concourse.mybir as mybir
from concourse.bass_utils import run_bass_kernel_spmd
from concourse.tile import TileContext

N_CORES = 8

# Graded configuration (reference.setup_inputs): n=12, target=5, B=4.
DIM = 4096
BATCH = 4
ROWS = DIM // N_CORES          # 512 rows per core
FREE = DIM * BATCH             # 16384 f32 per row
R_ROW = 64                     # rows right of target bit (row side)
RB = R_ROW * BATCH             # 256 f32: one col-side j-block
CGRP = 2 * RB                  # 512 f32: one col group (j=0 half + j=1 half)
W = 8192                       # chunk: f32 per partition per tile (16 col groups)
NW = FREE // W                 # 4 chunks
NG = ROWS // (4 * R_ROW)       # 2 supergroups of 256 rows (two 128-row a-groups)

_COEF_TOL = 0.0  # exact-zero test; bit-flip channel cross terms are exact 0s

_prog_cache: dict = {}


def _fused_groups(coefs, tol=1e-12):
    """Per-output-half term groups for the fused full-row scheme, or None.

    Folding the col-side q into full rows works iff each (i, j-pattern)
    group's coefficient is q-symmetric:
      aligned-diag (i=p,   j=q) / aligned-swap (i=p,   j=1-q)
      cross-diag   (i=1-p, j=q) / cross-swap   (i=1-p, j=1-q)
    and at most one cross group is nonzero (the ACT seed can't
    accumulate).  Bit-flip-like channels qualify.
    """

    def C(p, q, i, j):
        return coefs[((p * 2 + q) * 2 + i) * 2 + j]

    out = []
    for p in (0, 1):
        groups = []  # (coef, i, swap)
        for i in (0, 1):
            for swap in (False, True):
                c0 = C(p, 0, i, 1 if swap else 0)
                c1 = C(p, 1, i, 0 if swap else 1)
                if abs(c0 - c1) > tol:
                    return None
                if abs(c0) > tol:
                    groups.append((c0, i, swap))
        if len([g for g in groups if g[1] != p]) > 1:
            return None
        out.append(groups)
    return out


def _build_fused(
    coefs,
    repeat: int = 1,
    tile_w: int = 4096,
    bufs: int = 6,
    obufs: int = 6,
    taper: int = 0,
) -> "bass.Bass":
    """Fused full-row program: per [128, W] tile the whole update is
      2 seeds   (64-wide full rows; the cross term reads the other
                 partition half through a j-reversed view)
      1 STT     (128-wide full row, dense: the aligned-diag term)
    The second seed alternates ACT/DVE per tile to balance engine busy
    (~110 us each vs 147/149 for the blockwise program).
    """
    groups = _fused_groups(coefs)
    assert groups is not None

    f32 = mybir.dt.float32
    W_ = tile_w
    NW_ = FREE // W_
    n_agrp = ROWS // 128

    nc = bacc.Bacc("TRN2", target_bir_lowering=False, debug=False)
    x = nc.dram_tensor("x", [ROWS, FREE], f32, kind="ExternalInput")
    y = nc.dram_tensor("y", [ROWS, FREE], f32, kind="ExternalOutput")

    def v4(t):
        return t.rearrange("p (c j t) -> p c j t", j=2, t=RB)

    def src(xt, half, swap):
        s = v4(xt[half * 64 : (half + 1) * 64])
        return s[:, :, ::-1, :] if swap else s

    seeds = []  # (p, coef, i, swap)
    stts = []   # (p, coef, i, swap)  aligned accumulate
    for p in (0, 1):
        cross = [g for g in groups[p] if g[1] != p]
        aligned = [g for g in groups[p] if g[1] == p]
        if cross:
            seeds.append((p, *cross[0]))
            stts.extend((p, *g) for g in aligned)
        else:
            aligned = sorted(aligned, key=lambda g: -abs(g[0]))
            seeds.append((p, *aligned[0]))
            stts.extend((p, *g) for g in aligned[1:])

    diag = [s for s in stts if not s[3]]
    merge_diag = (
        len(diag) == 2
        and abs(diag[0][1] - diag[1][1]) < 1e-12
        and diag[0][0] != diag[1][0]
    )
    rest = [s for s in stts if s[3]] if merge_diag else stts

    from contextlib import ExitStack

    with TileContext(nc) as tc, ExitStack() as stack:
        if repeat > 1:
            stack.enter_context(tc.For_i(0, repeat, 1))
        with tc.tile_pool(name="xin", bufs=bufs) as px, \
             tc.tile_pool(name="yout", bufs=obufs) as po:

            def emit_compute(xt, ot, k):
                for idx, (p, coef, i, swap) in enumerate(seeds):
                    ov = v4(ot[p * 64 : (p + 1) * 64])
                    sv = src(xt, i, swap)
                    if idx == 1 and (k % 2):
                        nc.vector.tensor_scalar_mul(ov, sv, float(coef))
                    else:
                        nc.scalar.mul(ov, sv, float(coef))
                if merge_diag:
                    nc.vector.scalar_tensor_tensor(
                        out=ot,
                        in0=xt,
                        scalar=float(diag[0][1]),
                        in1=ot,
                        op0=mybir.AluOpType.mult,
                        op1=mybir.AluOpType.add,
                    )
                for p, coef, i, swap in rest:
                    ov = v4(ot[p * 64 : (p + 1) * 64])
                    nc.vector.scalar_tensor_tensor(
                        out=ov,
                        in0=src(xt, i, swap),
                        scalar=float(coef),
                        in1=ov,
                        op0=mybir.AluOpType.mult,
                        op1=mybir.AluOpType.add,
                    )

            k = 0
            for a in range(n_agrp):
                rs = slice(a * 128, (a + 1) * 128)
                for w in range(NW_):
                    cs = slice(w * W_, (w + 1) * W_)
                    xt = px.tile([128, W_], f32, tag="x")
                    nc.sync.dma_start(out=xt[:], in_=x[rs, cs])
                    ot = po.tile([128, W_], f32, tag="o")
                    last = a == n_agrp - 1 and w == NW_ - 1
                    if last and taper:
                        ns = 1 << taper
                        SW = W_ // ns
                        for s in range(ns):
                            ss = slice(s * SW, (s + 1) * SW)
                            emit_compute(xt[:, ss], ot[:, ss], k)
                            nc.sync.dma_start(
                                out=y[rs, w * W_ + s * SW : w * W_ + (s + 1) * SW],
                                in_=ot[:, ss],
                            )
                    else:
                        emit_compute(xt[:], ot[:], k)
                        nc.sync.dma_start(out=y[rs, cs], in_=ot[:])
                    k += 1
    nc.compile()
    return nc


def _build_best(coefs, repeat: int = 1) -> "bass.Bass":
    """Fused full-row program when the channel structure allows, else the
    generic blockwise program."""
    if _fused_groups(coefs) is not None:
        return _build_fused(coefs, repeat=repeat)
    return _build_program(coefs, repeat=repeat)


def _build_program(
    coefs: tuple,
    repeat: int = 1,
    tile_w: int = W,
    store_engine: str = "sync",
    bufs: int = 3,
    obufs: int | None = None,
    paired: bool = False,
) -> "bass.Bass":
    """Build the per-core SPMD program for coefficient matrix C[p,q,i,j].

    repeat > 1 wraps the whole body in a hardware loop — benchmarking only
    (recomputes the same output repeat times).
    """
    f32 = mybir.dt.float32
    W_ = tile_w
    NW_ = FREE // W_
    n_agrp = ROWS // 128  # natural 128-row groups per core

    nc = bacc.Bacc("TRN2", target_bir_lowering=False, debug=False)
    x = nc.dram_tensor("x", [ROWS, FREE], f32, kind="ExternalInput")
    y = nc.dram_tensor("y", [ROWS, FREE], f32, kind="ExternalOutput")

    def pjview(tile, p, j):
        # [64, ncg, RB]: partition half p (row target-bit), col-side j half
        # of every col group.
        return tile[p * 64 : (p + 1) * 64].rearrange(
            "p (c j t) -> p c j t", j=2, t=RB
        )[:, :, j, :]

    def terms_for(p, q):
        terms = [
            (coefs[((p * 2 + q) * 2 + i) * 2 + j], i, j)
            for i in (0, 1)
            for j in (0, 1)
            if abs(coefs[((p * 2 + q) * 2 + i) * 2 + j]) > _COEF_TOL
        ]
        terms.sort(key=lambda it: -abs(it[0]))
        return terms

    def emit_block(ov, xview, p, q, aligned_pred, scratch_view=None):
        # One ScalarE scaled copy seeds ov; remaining terms accumulate in
        # place via VectorE fused (x*c)+y.  In-place keeps each
        # instruction's semaphore-wait count low (the STT encoding has few
        # sync-wait slots).  HW constraint: STT's two SBUF inputs must
        # share a base partition, so terms whose source partition half
        # differs from ov's ("cross" terms) can only ride ScalarE (1-input,
        # cross-offset allowed) or accumulate in scratch at their own base.
        terms = terms_for(p, q)
        if not terms:
            nc.vector.memset(ov, 0.0)
            return
        aligned = [t for t in terms if aligned_pred(t[1])]
        cross = [t for t in terms if not aligned_pred(t[1])]
        if not cross:
            c0, i0, j0 = aligned[0]
            nc.scalar.mul(ov, xview(i0, j0), c0)
            rest = aligned[1:]
        elif len(cross) == 1:
            c0, i0, j0 = cross[0]
            nc.scalar.mul(ov, xview(i0, j0), c0)
            rest = aligned
        else:
            s = scratch_view(1 - p)
            c0, i0, j0 = cross[0]
            nc.scalar.mul(s, xview(i0, j0), c0)
            for ck, ik, jk in cross[1:]:
                nc.vector.scalar_tensor_tensor(
                    out=s,
                    in0=xview(ik, jk),
                    scalar=float(ck),
                    in1=s,
                    op0=mybir.AluOpType.mult,
                    op1=mybir.AluOpType.add,
                )
            nc.scalar.mul(ov, s, 1.0)
            rest = aligned
        for ck, ik, jk in rest:
            nc.vector.scalar_tensor_tensor(
                out=ov,
                in0=xview(ik, jk),
                scalar=float(ck),
                in1=ov,
                op0=mybir.AluOpType.mult,
                op1=mybir.AluOpType.add,
            )

    def jview128(tile, j):
        # [128, ncg, RB]: col-side j half of every col group, all partitions
        return tile.rearrange("p (c j t) -> p c j t", j=2, t=RB)[:, :, j, :]

    from contextlib import ExitStack

    if paired:
        with TileContext(nc) as tc, ExitStack() as stack:
            if repeat > 1:
                stack.enter_context(tc.For_i(0, repeat, 1))
            with tc.tile_pool(name="xin", bufs=bufs) as px, \
                 tc.tile_pool(name="yout", bufs=obufs or bufs) as po:
                for b in range(ROWS // 256):
                    r0 = b * 256
                    for w in range(NW_):
                        cs = slice(w * W_, (w + 1) * W_)
                        xt = []
                        for i in (0, 1):
                            t = px.tile([128, W_], f32, tag=f"x{i}")
                            nc.sync.dma_start(
                                out=t[0:64],
                                in_=x[r0 + i * 64 : r0 + i * 64 + 64, cs],
                            )
                            nc.sync.dma_start(
                                out=t[64:128],
                                in_=x[r0 + 128 + i * 64 : r0 + 128 + i * 64 + 64, cs],
                            )
                            xt.append(t)
                        for p in (0, 1):
                            ot = po.tile([128, W_], f32, tag=f"o{p}")
                            for q in (0, 1):
                                emit_block(
                                    jview128(ot, q),
                                    lambda i, j: jview128(xt[i], j),
                                    p,
                                    q,
                                    aligned_pred=lambda i: True,
                                )
                            eng = getattr(nc, store_engine)
                            eng.dma_start(
                                out=y[r0 + p * 64 : r0 + p * 64 + 64, cs],
                                in_=ot[0:64],
                            )
                            eng.dma_start(
                                out=y[r0 + 128 + p * 64 : r0 + 128 + p * 64 + 64, cs],
                                in_=ot[64:128],
                            )
        nc.compile()
        return nc

    # does any block route >=2 cross-partition terms through scratch?
    needs_scratch = any(
        len([t for t in terms_for(p, q) if t[1] != p]) >= 2
        for p in (0, 1)
        for q in (0, 1)
    )

    # scratch costs SBUF: drop to double buffering to stay within 224 KiB
    if needs_scratch:
        bufs = min(bufs, 2)

    with TileContext(nc) as tc, ExitStack() as stack:
        if repeat > 1:
            stack.enter_context(tc.For_i(0, repeat, 1))
        with tc.tile_pool(name="xin", bufs=bufs) as px, \
             tc.tile_pool(name="yout", bufs=obufs or bufs) as po, \
             tc.tile_pool(name="scr", bufs=2) as ps:
            for a in range(n_agrp):
                rs = slice(a * 128, (a + 1) * 128)
                for w in range(NW_):
                    cs = slice(w * W_, (w + 1) * W_)
                    xt = px.tile([128, W_], f32, tag="x")
                    # 128 consecutive DRAM rows -> 128 partitions; fully
                    # contiguous 32 KiB runs per partition (fast DMA path).
                    # Partitions 0-63 hold target-row-bit 0, 64-127 bit 1.
                    nc.sync.dma_start(out=xt[:], in_=x[rs, cs])
                    ot = po.tile([128, W_], f32, tag="o")
                    for p in (0, 1):
                        if needs_scratch:
                            st = ps.tile([128, W_ // 2], f32, tag="s")

                            def scratch_view(half, _st=st):
                                return _st[
                                    half * 64 : (half + 1) * 64
                                ].rearrange("p (c t) -> p c t", t=RB)
                        else:
                            scratch_view = None
                        for q in (0, 1):
                            # Reads with i != p are cross-partition-offset
                            # (supported on ScalarE; the STT same-base
                            # constraint is handled in emit_block).
                            emit_block(
                                pjview(ot, p, q),
                                lambda i, j: pjview(xt, i, j),
                                p,
                                q,
                                aligned_pred=lambda i, _p=p: i == _p,
                                scratch_view=scratch_view,
                            )
                    getattr(nc, store_engine).dma_start(
                        out=y[rs, cs], in_=ot[:]
                    )
    nc.compile()
    return nc


def _fallback(state, C, L, R, B):
    rho = state.reshape(L, 2, R, L, 2, R, B)
    out = np.einsum("pqij,aibcjdz->apbcqdz", C, rho.astype(np.float64))
    return out.reshape(state.shape).astype(state.dtype)


def kernel(state, kraus, target, n_qubits):
    state = np.asarray(state)
    kraus = np.asarray(kraus)
    t = int(np.asarray(target))
    n = int(np.asarray(n_qubits))
    dim = 1 << n
    B = state.shape[-1]
    L = 1 << t
    R = dim // (2 * L)

    C = np.einsum(
        "kpi,kqj->pqij",
        kraus.astype(np.float64),
        np.conj(kraus).astype(np.float64),
    )

    if not (
        state.shape == (DIM, DIM, BATCH)
        and state.dtype == np.float32
        and R == R_ROW
        and L * 2 * R == DIM
    ):
        return _fallback(state, C, L, R, B)

    coefs = tuple(float(v) for v in C.reshape(-1))
    nc = _prog_cache.get(coefs)
    if nc is None:
        nc = _build_best(coefs)
        _prog_cache[coefs] = nc

    flat = state.reshape(DIM, FREE)
    in_maps = [
        {"x": flat[c * ROWS : (c + 1) * ROWS]} for c in range(N_CORES)
    ]
    res = run_bass_kernel_spmd(nc, in_maps, core_ids=list(range(N_CORES)))
    out = np.concatenate([res.results[c]["y"] for c in range(N_CORES)], axis=0)
    return out.reshape(DIM, DIM, BATCH)

